# revision 1
# baseline (speedup 1.0000x reference)
"""Trainium2 Bass kernel: softmax((hidden@w1+b1) @ ((hidden+pre_emb)@w2+b2)^T).

Shapes: hidden/pre_emb [4, 4096, 1024], w1/w2 [1024,1024], b1/b2 [1024].
Output: [4, 4096, 4096] float32 (device writes bf16, host upcasts).

Algebraic fusion: softmax is invariant to adding a per-row constant, and
  (A@W1+b1) @ (g@W2+b2)^T = A@(W1@W2^T)@g^T + 1*(g@(W2@b1))^T + row-consts
with g = hidden+pre_emb.  With M = W1@W2^T and v = W2@b1 precomputed on-chip
(~75k PE cycles incl. weight transposes), scores' = (A@M + v) @ g^T needs NO
key-side projection at all -- the key operand is just transpose(hidden+pre),
summed by a DMA-accumulate.  This removes the baseline's 262k-cycle proj2
(which was also computed redundantly by both cores of a batch pair) and cuts
per-core PE work ~1.05M -> ~830k cycles.

Sharding: 8 cores = 4 batches x 2 query-halves (2048 q x 4096 keys each).
H = A_q@M+v ([1024e x 2048q] f32r, 8MB) is built once and stays resident in
SBUF for both key rounds (the baseline's 16MB DRAM hT spill/reload is gone).
Keys run in 2 rounds of 2048 with a flash-softmax merge; round 0 spills
unnormalized exp as bf16 to a DRAM scratch, round 1 merges stats, writes its
half, and rescales the scratch half.  The output is written in bf16 (softmax
probs; adds ~1e-3 rel err) halving output DMA to 16MB.

Engine notes: all matmuls f32r (1 cyc/row at free>=256); transposes f32r
(1.5 cyc/row -- load tiles are DMA'd as f32r bits, legal since the verifier
only requires engine-produced f32r for rounding), batched 2-per-PSUM bank;
block maxes reduced directly from PSUM; per-engine DMA queues: sync HWDGE
for A-half0/weights/scratch/out, scalar SWDGE for A-half1, gpsimd SWDGE for
P-accumulate + fixup writes.  gT strips live in a 4-deep ring per e-block so
round-1 strips build during round-0 scores / interleaved into qb0's matmuls.
Per-core DMA ~80MB; cost-model sim 420us/core vs 571us for the baseline
(measured HW baseline 422us; sim tracks ~0.75x).
"""

import numpy as np

import concourse.bass as bass
import concourse.tile as tile
from concourse import bacc, masks, mybir
from concourse.bass_utils import run_bass_kernel_spmd

F32 = mybir.dt.float32
BF16 = mybir.dt.bfloat16
F32R = mybir.dt.float32r
AF = mybir.ActivationFunctionType
ALU = mybir.AluOpType

B, S, D = 4, 4096, 1024
QP = S // 2          # queries per core = 2048
N_CORES = 8
KH = S // 2          # keys per round = 2048

_cache = {}
TRACE = False
LAST_EXEC_NS = None


def _build():
    if "nc" in _cache:
        return _cache["nc"]

    nc = bacc.Bacc("TRN2", target_bir_lowering=False, debug=False,
                   enable_asserts=False, num_devices=N_CORES)

    hid_q = nc.dram_tensor("hid_q", [QP, D], F32, kind="ExternalInput").ap()
    hid_kv = nc.dram_tensor("hid_kv", [S, D], F32, kind="ExternalInput").ap()
    pre_kv = nc.dram_tensor("pre_kv", [S, D], F32, kind="ExternalInput").ap()
    w1_d = nc.dram_tensor("w1", [D, D], F32, kind="ExternalInput").ap()
    w2_d = nc.dram_tensor("w2", [D, D], F32, kind="ExternalInput").ap()
    b1_d = nc.dram_tensor("b1", [D], F32, kind="ExternalInput").ap()
    out_d = nc.dram_tensor("out", [QP, S], BF16, kind="ExternalOutput").ap()

    from contextlib import ExitStack
    with tile.TileContext(nc) as tc:
        itctx = ExitStack()   # iT strip ring, live to the end
        mctx = ExitStack()    # M tiles (above it on the pool stack)
        wmctx = ExitStack()   # W1T/W2T tiles, freed after M
        endctx = ExitStack()  # hkeep (H tiles), created after wm pops
        qtctx = ExitStack()   # qT strips, freed after H build
        fbctx = ExitStack()   # score-phase pools, created after H build
        with tc.tile_pool(name="consts", bufs=1) as consts, \
             tc.tile_pool(name="loads", bufs=3) as loads, \
             tc.tile_pool(name="keep", bufs=1) as keep, \
             tc.tile_pool(name="st", bufs=2) as stpool, \
             tc.tile_pool(name="dram", bufs=1, space="DRAM") as dpool, \
             tc.tile_pool(name="pstr", bufs=3, space="PSUM") as pstr, \
             tc.tile_pool(name="ppr", bufs=2, space="PSUM") as ppr, \
             tc.tile_pool(name="psc", bufs=3, space="PSUM") as psc:

            identr = consts.tile([128, 128], F32R)
            # b1 laid out [128, 8]: column fi = b1[fi*128:(fi+1)*128]
            b1c = consts.tile([128, 8], F32)
            nc.sync.dma_start(b1c[:], b1_d.rearrange("(a b) -> b a", a=8))
            b1r = consts.tile([128, 8], F32R)
            nc.vector.tensor_copy(b1r[:], b1c[:])
            vt = consts.tile([128, 8], F32)   # v = W2@b1, col mo = e-block

            act_copy = nc.scalar.copy
            vec_copy = nc.vector.tensor_copy

            def tr2(dst_ap, src0, src1, eng):
                # two 128x128 transposes into one [128,256] psum + 1 copy
                # (copy casts f32 -> f32r; f32r-tagged transpose inputs are
                # rejected by the BIR verifier for DMA-produced data)
                tp = pstr.tile([128, 256], F32R, tag="tr", name="tp")
                nc.tensor.transpose(tp[:, 0:128], src0, identr[:])
                nc.tensor.transpose(tp[:, 128:256], src1, identr[:])
                eng(dst_ap, tp[:])

            itpool = itctx.enter_context(tc.tile_pool(name="it", bufs=4))
            iT_strips = {}

            def alloc_gT_strip(rnd, kt):
                sts = [itpool.tile([128, 512], F32R, tag=f"it{mo}",
                                   name=f"it{rnd}_{kt}_{mo}") for mo in range(8)]
                iT_strips[(rnd, kt)] = sts
                return sts

            def build_gT_half(rnd, kt, half):
                sts = iT_strips[(rnd, kt)]
                r0 = rnd * KH + kt * 512 + half * 256
                lt = loads.tile([128, 2 * D], F32R, tag="load",
                                name=f"lt{rnd}_{kt}_{half}")
                qdma = nc.sync.dma_start if half == 0 else nc.scalar.dma_start
                qdma(lt[:], hid_kv[r0:r0 + 256, :].rearrange(
                    "(j p) c -> p j c", p=128).bitcast(F32R))
                nc.gpsimd.dma_start(
                    lt[:], pre_kv[r0:r0 + 256, :].rearrange(
                        "(j p) c -> p j c", p=128).bitcast(F32R),
                    accum_op=ALU.add)
                for ki in range(8):
                    tr2(sts[ki][:, half * 256:(half + 1) * 256],
                        lt[:, ki * 128:(ki + 1) * 128],
                        lt[:, D + ki * 128:D + (ki + 1) * 128],
                        act_copy if ki % 2 == 0 else vec_copy)

            def build_gT_strip(rnd, kt):
                alloc_gT_strip(rnd, kt)
                build_gT_half(rnd, kt, 0)
                build_gT_half(rnd, kt, 1)

            # ---- load W1 (sync) / W2 (gpsimd) in parallel; build W1T/W2T,
            # interleaving the first two key strips to keep the PE fed ----
            mpool = mctx.enter_context(tc.tile_pool(name="m", bufs=1))
            Mr = [mpool.tile([128, D], F32R, tag=f"m{ki}", name=f"m{ki}")
                  for ki in range(8)]
            wmpool = wmctx.enter_context(tc.tile_pool(name="wm", bufs=1))
            identf = wmpool.tile([128, 128], F32, tag="idf", name="identf")
            masks.make_identity(nc, identf[:])
            nc.vector.tensor_copy(identr[:], identf[:])
            w1T = [wmpool.tile([128, D], F32R, tag=f"w1T{fi}", name=f"w1T{fi}")
                   for fi in range(8)]
            w2T = [wmpool.tile([128, D], F32R, tag=f"w2T{fi}", name=f"w2T{fi}")
                   for fi in range(8)]
            for ch in range(4):
                for wn, (wd, wT) in enumerate(((w1_d, w1T), (w2_d, w2T))):
                    lt = loads.tile([128, 2 * D], F32R, tag="load",
                                    name=f"w{wn}_{ch}")
                    dma = nc.sync.dma_start if wn == 0 else nc.gpsimd.dma_start
                    dma(lt[:], wd[ch * 256:(ch + 1) * 256, :].rearrange(
                        "(j p) c -> p j c", p=128).bitcast(F32R))
                    for fi in range(8):
                        tr2(wT[fi][:, ch * 256:(ch + 1) * 256],
                            lt[:, fi * 128:(fi + 1) * 128],
                            lt[:, D + fi * 128:D + (fi + 1) * 128],
                            act_copy if fi % 2 == wn else vec_copy)

            # ---- v = W2 @ b1: row [1,1024] via b1-stationary matmuls,
            # then a DRAM roundtrip to per-partition [128, 8] bias layout ----
            vrow_d = dpool.tile([D], F32, name="vrow_d")
            vs = wmpool.tile([1, D], F32, tag="vs", name="vs")
            for half in range(2):
                vp = ppr.tile([128, 512], F32, tag="pr", name=f"vp{half}")
                for fi in range(8):
                    nc.tensor.matmul(vp[0:1, :], b1r[:, fi:fi + 1],
                                     w2T[fi][:, half * 512:(half + 1) * 512],
                                     start=(fi == 0), stop=(fi == 7))
                nc.scalar.copy(vs[0:1, half * 512:(half + 1) * 512], vp[0:1, :])
            nc.sync.dma_start(vrow_d[:], vs[0:1, 0:D])
            nc.sync.dma_start(vt[:], vrow_d[:].rearrange("(a b) -> b a", a=8))

            # ---- M = W1 @ W2^T: 8 lhsT tiles [128 d, 1024 e] f32r ----
            for ki in range(8):
                for half in range(2):
                    ps = ppr.tile([128, 512], F32, tag="pr",
                                  name=f"mp{ki}_{half}")
                    for fi in range(8):
                        nc.tensor.matmul(ps[:],
                                         w1T[fi][:, ki * 128:(ki + 1) * 128],
                                         w2T[fi][:, half * 512:(half + 1) * 512],
                                         start=(fi == 0), stop=(fi == 7))
                    (act_copy if (ki + half) % 2 == 0 else vec_copy)(
                        Mr[ki][:, half * 512:(half + 1) * 512], ps[:])
            wmctx.close()

            hkeep = endctx.enter_context(tc.tile_pool(name="hkeep", bufs=1))
            strips = qtctx.enter_context(tc.tile_pool(name="qt", bufs=1))

            # ---- H = A_q@M + v, resident [128 e, 512 q] x8x4 f32r
            # (per-qc tiles so early scores don't wait on later qc writes) ----
            hT = {(mo, qc): hkeep.tile([128, 512], F32R, tag=f"h{mo}_{qc}",
                                       name=f"h{mo}_{qc}")
                  for mo in range(8) for qc in range(4)}

            def build_H_qc(qc):
                qTs = [strips.tile([128, 512], F32R, tag=f"qt{ki}",
                                   name=f"qt{qc}_{ki}") for ki in range(8)]
                for half in range(2):
                    r0 = qc * 512 + half * 256
                    hq = loads.tile([128, 2 * D], F32R, tag="load",
                                    name=f"hq{qc}_{half}")
                    qdma = nc.sync.dma_start if half == 0 else nc.scalar.dma_start
                    qdma(
                        hq[:], hid_q[r0:r0 + 256, :].rearrange(
                            "(j p) c -> p j c", p=128).bitcast(F32R))
                    for ki in range(8):
                        tr2(qTs[ki][:, half * 256:(half + 1) * 256],
                            hq[:, ki * 128:(ki + 1) * 128],
                            hq[:, D + ki * 128:D + (ki + 1) * 128],
                            act_copy if ki % 2 == 1 else vec_copy)
                for mo in range(8):
                    ps = ppr.tile([128, 512], F32, tag="pr",
                                  name=f"hp{qc}_{mo}")
                    for ki in range(8):
                        nc.tensor.matmul(ps[:],
                                         Mr[ki][:, mo * 128:(mo + 1) * 128],
                                         qTs[ki][:],
                                         start=(ki == 0), stop=(ki == 7))
                    nc.scalar.activation(hT[(mo, qc)][:], ps[:],
                                         AF.Identity, bias=vt[:, mo:mo + 1])

            # interleave round-0 gT strips with H so PE stays fed while
            # the key chunks stream in
            for step in range(4):
                build_gT_strip(0, step)
                build_H_qc(step)
            qtctx.close()
            scpool = fbctx.enter_context(tc.tile_pool(name="sc", bufs=3))
            scbpool = fbctx.enter_context(tc.tile_pool(name="scb", bufs=3))
            fbpool = fbctx.enter_context(tc.tile_pool(name="fb", bufs=2))

            scratch = dpool.tile([QP, KH], BF16)
            # per-qb saved stats from round 0: cols [2*qb]=rowmax, [2*qb+1]=rowsum
            svt = keep.tile([128, 32], F32, name="svt", tag="svt")
            saved = [svt[:, 2 * i:2 * i + 2] for i in range(16)]

            for rnd in range(2):
                if rnd == 1:
                    build_gT_strip(1, 0)
                iT = iT_strips

                for qb in range(16):
                    q0 = qb * 128
                    blockmax = stpool.tile([128, 4], F32, tag="bm")
                    sums = stpool.tile([128, 2], F32, tag="sm")
                    blocks = []
                    if rnd == 1:
                        # prefetch round-0 scratch for the fixup while this
                        # qb's scores are still on the PE
                        pre_fbb = []
                        for kh in range(2):
                            fbb = fbpool.tile([128, 1024], BF16, tag="fbb",
                                              name=f"fbb{qb}_{kh}")
                            nc.sync.dma_start(
                                fbb[:],
                                scratch[q0:q0 + 128,
                                        kh * 1024:(kh + 1) * 1024])
                            pre_fbb.append(fbb)
                    for kh in range(2):
                        blk = scpool.tile([128, 1024], F32, tag="blk",
                                          name=f"blk{rnd}_{qb}_{kh}")
                        for sub in range(2):
                            kb = kh * 2 + sub
                            if rnd == 1 and qb == 0 and kb >= 1:
                                build_gT_strip(1, kb)
                            ps = psc.tile([128, 512], F32, tag="ps", name="ps")
                            hq0 = (qb % 4) * 128
                            for mo in range(8):
                                nc.tensor.matmul(
                                    ps[:],
                                    hT[(mo, qb // 4)][:, hq0:hq0 + 128],
                                    iT[(rnd, kb)][mo][:],
                                    start=(mo == 0), stop=(mo == 7))
                            ceng = act_copy if (sub == 0 or rnd == 1) \
                                else vec_copy
                            ceng(blk[:, sub * 512:(sub + 1) * 512], ps[:])
                            nc.vector.tensor_reduce(blockmax[:, kb:kb + 1],
                                                    ps[:],
                                                    axis=mybir.AxisListType.X,
                                                    op=ALU.max)
                        blocks.append(blk)
                    if rnd == 0:
                        rowmax = saved[qb][:, 0:1]
                    else:
                        rowmax = stpool.tile([128, 1], F32, tag="rm")
                    nc.vector.tensor_reduce(rowmax[:], blockmax[:],
                                            axis=mybir.AxisListType.X,
                                            op=ALU.max)
                    negmax = stpool.tile([128, 1], F32, tag="nm")
                    nc.vector.tensor_scalar_mul(negmax[:], rowmax[:], -1.0)
                    if rnd == 1:
                        m1 = saved[qb][:, 0:1]
                        s1 = saved[qb][:, 1:2]
                        # negm = -max(m1, rowmax); e1/e2 emitted before the
                        # big block exps so the fixup chain starts early
                        negm = stpool.tile([128, 1], F32, tag="ngm")
                        nc.vector.tensor_scalar(negm[:], rowmax[:], m1, -1.0,
                                                op0=ALU.max, op1=ALU.mult)
                        e1 = stpool.tile([128, 1], F32, tag="e1")
                        nc.scalar.activation(e1[:], m1, AF.Exp, bias=negm[:])
                        e2 = stpool.tile([128, 1], F32, tag="e2")
                        nc.scalar.activation(e2[:], rowmax[:], AF.Exp,
                                             bias=negm[:])
                        t1 = stpool.tile([128, 1], F32, tag="t1")
                        nc.vector.tensor_tensor(t1[:], s1, e1[:], op=ALU.mult)
                    bfb = []
                    for kh in range(2):
                        if rnd == 0:
                            bb = scbpool.tile([128, 1024], BF16, tag="blkb",
                                              name=f"bb{qb}_{kh}")
                            nc.scalar.activation(bb[:], blocks[kh][:], AF.Exp,
                                                 bias=negmax[:],
                                                 accum_out=sums[:, kh:kh + 1])
                            bfb.append(bb)
                        else:
                            eb = scbpool.tile([128, 1024], BF16, tag="blkb",
                                              name=f"eb{qb}_{kh}")
                            nc.scalar.activation(eb[:], blocks[kh][:],
                                                 AF.Exp, bias=negmax[:],
                                                 accum_out=sums[:, kh:kh + 1])
                            bfb.append(eb)
                    if rnd == 0:
                        rowsum = saved[qb][:, 1:2]
                    else:
                        rowsum = stpool.tile([128, 1], F32, tag="rs")
                    nc.vector.tensor_reduce(rowsum[:], sums[:],
                                            axis=mybir.AxisListType.X,
                                            op=ALU.add)
                    if rnd == 0:
                        for kh in range(2):
                            nc.sync.dma_start(
                                scratch[q0:q0 + 128,
                                        kh * 1024:(kh + 1) * 1024],
                                bfb[kh][:])
                    else:
                        # z = s2*e2 + (s1*e1)
                        z = stpool.tile([128, 1], F32, tag="z")
                        nc.vector.scalar_tensor_tensor(z[:], rowsum[:], e2[:],
                                                       t1[:], op0=ALU.mult,
                                                       op1=ALU.add)
                        rz = stpool.tile([128, 1], F32, tag="rz")
                        nc.vector.reciprocal(rz[:], z[:])
                        r1 = stpool.tile([128, 1], F32, tag="r1")
                        nc.vector.tensor_tensor(r1[:], e1[:], rz[:],
                                                op=ALU.mult)
                        r2 = stpool.tile([128, 1], F32, tag="r2")
                        nc.vector.tensor_tensor(r2[:], e2[:], rz[:],
                                                op=ALU.mult)
                        for kh in range(2):
                            nc.vector.tensor_scalar_mul(bfb[kh][:],
                                                        bfb[kh][:], r2[:])
                            nc.sync.dma_start(
                                out_d[q0:q0 + 128,
                                      KH + kh * 1024:KH + (kh + 1) * 1024],
                                bfb[kh][:])
                        # rescale round-0 half from scratch (bf16 in/out)
                        for kh in range(2):
                            fbf = scbpool.tile([128, 1024], BF16, tag="blkb",
                                               name=f"fbf{qb}_{kh}")
                            nc.vector.tensor_scalar_mul(fbf[:],
                                                        pre_fbb[kh][:], r1[:])
                            nc.gpsimd.dma_start(
                                out_d[q0:q0 + 128,
                                      kh * 1024:(kh + 1) * 1024],
                                fbf[:])

            fbctx.close()
            endctx.close()
            mctx.close()
            itctx.close()

    nc.compile()
    _cache["nc"] = nc
    return nc


def make_in_maps(np_inputs):
    hidden = np_inputs["hidden"]
    pre_emb = np_inputs["pre_emb"]
    in_maps = []
    for c in range(N_CORES):
        b, qh = c // 2, c % 2
        in_maps.append({
            "hid_q": np.ascontiguousarray(hidden[b, qh * QP:(qh + 1) * QP, :]),
            "hid_kv": np.ascontiguousarray(hidden[b]),
            "pre_kv": np.ascontiguousarray(pre_emb[b]),
            "w1": np_inputs["w1"], "w2": np_inputs["w2"],
            "b1": np_inputs["b1"],
        })
    return in_maps


def kernel(hidden, pre_emb, w1, b1, w2, b2):
    hidden = np.ascontiguousarray(np.asarray(hidden, dtype=np.float32))
    pre_emb = np.ascontiguousarray(np.asarray(pre_emb, dtype=np.float32))
    w1 = np.ascontiguousarray(np.asarray(w1, dtype=np.float32))
    b1 = np.ascontiguousarray(np.asarray(b1, dtype=np.float32))
    w2 = np.ascontiguousarray(np.asarray(w2, dtype=np.float32))

    nc = _build()
    in_maps = []
    for c in range(N_CORES):
        b, qh = c // 2, c % 2
        in_maps.append({
            "hid_q": np.ascontiguousarray(hidden[b, qh * QP:(qh + 1) * QP, :]),
            "hid_kv": hidden[b],
            "pre_kv": pre_emb[b],
            "w1": w1, "w2": w2, "b1": b1,
        })
    kw = {}
    if TRACE:
        kw = dict(trace=True, trace_cores=[0])
    res = run_bass_kernel_spmd(nc, in_maps, core_ids=list(range(N_CORES)), **kw)
    global LAST_EXEC_NS
    if res.exec_time_ns is not None:
        LAST_EXEC_NS = res.exec_time_ns
    out = np.empty((B, S, S), dtype=np.float32)
    for c in range(N_CORES):
        b, qh = c // 2, c % 2
        out[b, qh * QP:(qh + 1) * QP, :] = res.results[c]["out"]
    return out

